# revision 1
# baseline (speedup 1.0000x reference)
"""Causal self-attention (B=4, S=2048, D=768, H=12) on 8 TRN2 NeuronCores.

Sharding: batch (4) x head-group (2) = 8 cores.  Each core computes, for its
batch b and 6 heads:
  - x^T via PE transposes (contraction over D needs D on partitions)
  - QK^T projection directly in transposed layout (head-dim on partitions),
    Q pre-scaled by 1/sqrt(dh) via host-side weight scaling
  - V projection in natural layout, with a ones column appended per head
    (so the AV matmul also produces softmax denominators for free)
  - flash-style causal attention with scores kept transposed
    (S^T = K Q^T): softmax needs no max-subtraction (scores are O(1) here),
    exp on ACT, causal mask as a 0/1 multiply on diagonal blocks only
  - AV^T accumulated in PSUM over key chunks -> O^T [dh, q] per head,
    normalized by PE-broadcast reciprocal of the fused sums row
  - partial output projection (its 384 rows of W_out)
Host: sums the two partial outputs per batch and adds the constant
b_v @ W_out + b_out (V-bias commutes through softmax-normalized attention).

All matmuls run in float32r (full-rate fp32 on the PE, ~1.2e-4 rounding).
"""

import numpy as np

import concourse.bass as bass
import concourse.tile as tile
import concourse.mybir as mybir
from concourse import bacc
from concourse._compat import with_exitstack  # noqa: F401  (parity with repo kernels)

F32 = mybir.dt.float32
F32R = mybir.dt.float32r

B, S, D = 4, 2048, 768
H, DH = 12, 64
G = 2                 # head groups (tensor-parallel dimension)
HPG = H // G          # heads per group = 6
NPAIR = HPG // 2      # head pairs per group = 3
N_CORES = 8
ST = 128              # S-tile for projections / output rows
QT = 512              # q-tile for attention
KC = 128              # key chunk
N_ST = S // ST        # 16
N_QT = S // QT        # 4
DC = D // 128         # 6 contraction chunks over D


def declare_io(nc):
    """DRAM tensors; names must match in_maps keys."""
    io = {}
    io["x"] = nc.dram_tensor("x", [S, D], F32R, kind="ExternalInput")
    io["wqk"] = nc.dram_tensor("wqk", [D, 768], F32R, kind="ExternalInput")
    io["bqk2"] = nc.dram_tensor("bqk2", [128, 6], F32, kind="ExternalInput")
    io["wv"] = nc.dram_tensor("wv", [D, 384], F32R, kind="ExternalInput")
    io["wo"] = nc.dram_tensor("wo", [384, 768], F32R, kind="ExternalInput")
    io["masks"] = nc.dram_tensor("masks", [2, KC, QT], F32R, kind="ExternalInput")
    io["ident"] = nc.dram_tensor("ident", [128, 128], F32R, kind="ExternalInput")
    io["sel"] = nc.dram_tensor("sel", [128, 128], F32R, kind="ExternalInput")
    io["ones2"] = nc.dram_tensor("ones2", [128, HPG], F32R, kind="ExternalInput")
    io["out"] = nc.dram_tensor("out", [S, D], F32, kind="ExternalOutput")
    return io


def build_body(nc, tc, pools, io, phases=(1, 2, 3, 4, 5)):
    """Emit one full forward pass (per-core program)."""
    (consts, w768, wsmall, slab, qkT_p, vsb_p, xload, psA, psB, scp, pT_p,
     rcp_p, atmp_p, outsb_p) = pools

    # ---- constants / weights into SBUF ----
    ident_t = consts.tile([128, 128], F32R, tag="ident")
    nc.sync.dma_start(out=ident_t, in_=io["ident"][:])
    sel_t = consts.tile([128, 128], F32R, tag="sel")
    nc.sync.dma_start(out=sel_t, in_=io["sel"][:])
    bqk2_t = consts.tile([128, 6], F32, tag="bqk2")
    nc.sync.dma_start(out=bqk2_t, in_=io["bqk2"][:])
    masks_t = []
    for r in range(2):
        m = consts.tile([KC, QT], F32R, tag=f"mask{r}")
        nc.sync.dma_start(out=m, in_=io["masks"][r])
        masks_t.append(m)

    wqk_t = []
    for c in range(DC):
        w = w768.tile([128, 768], F32R, tag="w768")
        nc.sync.dma_start(out=w, in_=io["wqk"][c * 128:(c + 1) * 128, :])
        wqk_t.append(w)
    wv_t = []
    for c in range(DC):
        w = wsmall.tile([128, 384], F32R, tag="wv")
        nc.sync.dma_start(out=w, in_=io["wv"][c * 128:(c + 1) * 128, :])
        wv_t.append(w)

    # ---- x^T (6 tiles [128, S]) via PE transposes, 4 S-tiles per copy ----
    xT = [slab.tile([128, S], F32R, tag="slab", name=f"xT{c}") for c in range(DC)]
    for s4 in range(N_ST // 4):
        xts = []
        for si in range(4):
            s = s4 * 4 + si
            xt = xload.tile([128, D], F32R, tag="xload")
            nc.sync.dma_start(out=xt, in_=io["x"][s * ST:(s + 1) * ST, :])
            xts.append(xt)
        for c in range(DC):
            tp = psA.tile([128, 512], F32R, tag="ps1")
            for si in range(4):
                nc.tensor.transpose(tp[:, si * 128:(si + 1) * 128],
                                    xts[si][:, c * 128:(c + 1) * 128], ident_t[:])
            nc.vector.tensor_copy(xT[c][:, s4 * 512:(s4 + 1) * 512], tp)

    if 2 not in phases:
        return
    # ---- QK^T projection: qkT[j] j even = Q-pair j//2, j odd = K-pair ----
    qkT = [qkT_p.tile([128, S], F32R, tag="qkT", name=f"qkT{j}") for j in range(6)]
    for j in range(6):
        for t in range(N_QT):
            pp = psA.tile([128, QT], F32, tag="ps1")
            for c in range(DC):
                nc.tensor.matmul(pp, wqk_t[c][:, j * 128:(j + 1) * 128],
                                 xT[c][:, t * QT:(t + 1) * QT],
                                 start=(c == 0), stop=(c == DC - 1))
            nc.vector.tensor_scalar_add(qkT[j][:, t * QT:(t + 1) * QT], pp,
                                        bqk2_t[:, j:j + 1])

    if 3 not in phases:
        return
    # ---- V projection into [V_h | ones] blocks of 65 cols ----
    vsb = []
    for s in range(N_ST):
        vp = psA.tile([128, 384], F32, tag="ps1")
        for c in range(DC):
            nc.tensor.matmul(vp, xT[c][:, s * ST:(s + 1) * ST], wv_t[c][:],
                             start=(c == 0), stop=(c == DC - 1))
        vv = vsb_p.tile([128, HPG, 65], F32R, tag="vsb")
        nc.vector.tensor_copy(vv[:, :, 0:64],
                              vp[:].rearrange("p (h d) -> p h d", h=HPG))
        nc.sync.dma_start(out=vv[:, :, 64:65],
                          in_=io["ones2"][:].rearrange("p (h o) -> p h o", o=1))
        vsb.append(vv)

    if 4 not in phases:
        return
    # ---- attention ----
    apair = [slab.tile([128, S], F32R, tag="slab", name=f"apair{p}") for p in range(NPAIR)]
    for p in range(NPAIR):
        qp = qkT[2 * p]
        kp = qkT[2 * p + 1]
        for t in range(N_QT):
            n_kc = 4 * t + 4
            av_e = psB.tile([65, QT], F32, tag="ps2")
            av_o = psB.tile([65, QT], F32, tag="ps2")
            avs = (av_e, av_o)

            def chunk_geom(kc):
                # causal slicing: diagonal chunk r only touches q-window
                # [off, 512); r==3 widened to 256 so fp32r stays full-rate.
                r = kc - 4 * t
                if r < 0:
                    return 0, QT, None
                if r < 3:
                    return 128 * r, QT - 128 * r, masks_t[0][:, 0:128]
                return 256, 256, masks_t[1][:, 0:256]

            for kc in range(n_kc):
                off, w, msk = chunk_geom(kc)
                # both heads' scores in one 2-bank PSUM tile so exp and the
                # causal-mask multiply run once per chunk pair (ACT per-op
                # overhead is ~300ns; halving the op count matters)
                sc2 = scp.tile([KC, 2, QT], F32, tag="sc2")
                pt2 = pT_p.tile([KC, 2, QT], F32R, tag="pT")
                for j in (0, 1):
                    nc.tensor.matmul(
                        sc2[:, j, 0:w],
                        kp[j * 64:(j + 1) * 64, kc * KC:(kc + 1) * KC],
                        qp[j * 64:(j + 1) * 64, t * QT + off:(t + 1) * QT],
                        start=True, stop=True, tile_position=(j * 64, 0))
                nc.scalar.activation(pt2[:, :, 0:w], sc2[:, :, 0:w],
                                     mybir.ActivationFunctionType.Exp)
                if msk is not None:
                    mw = msk.shape[1]
                    msk2 = bass.AP(tensor=msk.tensor, offset=msk.offset,
                                   ap=[list(msk.ap[0]), [0, 2], list(msk.ap[1])])
                    nc.vector.tensor_mul(pt2[:, :, 0:mw], pt2[:, :, 0:mw], msk2)
                for j in (0, 1):
                    nc.tensor.matmul(
                        avs[j][:, off:off + w], vsb[kc][:, 2 * p + j, :],
                        pt2[:, j, 0:w],
                        start=(kc == 0), stop=(kc == n_kc - 1))
            rc_e = rcp_p.tile([65, QT], F32R, tag="rcp")
            nc.vector.reciprocal(rc_e[64:65, :], av_e[64:65, :])
            rc_o = rcp_p.tile([65, QT], F32R, tag="rcp")
            nc.vector.reciprocal(rc_o[64:65, :], av_o[64:65, :])
            bc_e = psA.tile([64, QT], F32, tag="ps1")
            nc.tensor.matmul(bc_e, sel_t[64:65, 0:64], rc_e[64:65, :],
                             start=True, stop=True)
            bc_e_sb = rcp_p.tile([64, QT], F32, tag="bcsb")
            nc.vector.tensor_copy(bc_e_sb, bc_e)
            bc_o = psA.tile([64, QT], F32, tag="ps1")
            nc.tensor.matmul(bc_o, sel_t[64:65, 0:64], rc_o[64:65, :],
                             start=True, stop=True)
            bc_o_sb = rcp_p.tile([64, QT], F32, tag="bcsb")
            nc.vector.tensor_copy(bc_o_sb, bc_o)
            nc.vector.tensor_mul(apair[p][0:64, t * QT:(t + 1) * QT],
                                 av_e[0:64, :], bc_e_sb[:])
            at = atmp_p.tile([64, QT], F32R, tag="atmp")
            nc.vector.tensor_mul(at, av_o[0:64, :], bc_o_sb[:])
            nc.sync.dma_start(out=apair[p][64:128, t * QT:(t + 1) * QT], in_=at)

    if 5 not in phases:
        return
    # ---- output projection (partial: this group's 384 rows of W_out) ----
    wo_t = []
    for p in range(NPAIR):
        w = w768.tile([128, 768], F32R, tag="w768")
        nc.sync.dma_start(out=w, in_=io["wo"][p * 128:(p + 1) * 128, :])
        wo_t.append(w)
    for s in range(N_ST):
        o1 = psA.tile([128, 512], F32, tag="ps1")
        o2 = psA.tile([128, 256], F32, tag="ps1")
        for p in range(NPAIR):
            nc.tensor.matmul(o1, apair[p][:, s * ST:(s + 1) * ST],
                             wo_t[p][:, 0:512],
                             start=(p == 0), stop=(p == NPAIR - 1))
        for p in range(NPAIR):
            nc.tensor.matmul(o2, apair[p][:, s * ST:(s + 1) * ST],
                             wo_t[p][:, 512:768],
                             start=(p == 0), stop=(p == NPAIR - 1))
        osb = outsb_p.tile([128, D], F32, tag="outsb")
        nc.vector.tensor_copy(osb[:, 0:512], o1)
        nc.vector.tensor_copy(osb[:, 512:768], o2)
        nc.sync.dma_start(out=io["out"][s * ST:(s + 1) * ST, :], in_=osb)


def make_pools(tc, ctx):
    consts = ctx.enter_context(tc.tile_pool(name="consts", bufs=1))
    w768 = ctx.enter_context(tc.tile_pool(name="w768", bufs=6))
    wsmall = ctx.enter_context(tc.tile_pool(name="wsmall", bufs=6))
    slab = ctx.enter_context(tc.tile_pool(name="slab", bufs=6))
    qkT_p = ctx.enter_context(tc.tile_pool(name="qkT", bufs=6))
    vsb_p = ctx.enter_context(tc.tile_pool(name="vsb", bufs=16))
    xload = ctx.enter_context(tc.tile_pool(name="xload", bufs=5))
    psA = ctx.enter_context(tc.tile_pool(name="psA", bufs=2, space="PSUM"))
    psB = ctx.enter_context(tc.tile_pool(name="psB", bufs=2, space="PSUM"))
    scp = ctx.enter_context(tc.tile_pool(name="scp", bufs=2, space="PSUM"))
    pT_p = ctx.enter_context(tc.tile_pool(name="pT", bufs=3))
    rcp_p = ctx.enter_context(tc.tile_pool(name="rcp", bufs=2))
    atmp_p = ctx.enter_context(tc.tile_pool(name="atmp", bufs=2))
    outsb_p = ctx.enter_context(tc.tile_pool(name="outsb", bufs=2))
    return (consts, w768, wsmall, slab, qkT_p, vsb_p, xload, psA, psB, scp,
            pT_p, rcp_p, atmp_p, outsb_p)


def build_nc(n_iters=None, phases=(1, 2, 3, 4, 5)):
    """Build the per-core program. n_iters wraps the body in a HW loop
    (timing harness only; the graded path uses n_iters=None)."""
    from contextlib import ExitStack

    nc = bacc.Bacc(trn_type="TRN2", debug=False)
    nc._allow_low_precision_reason = "float32r matmuls keep fp32 width"
    io = declare_io(nc)
    with tile.TileContext(nc) as tc:
        with ExitStack() as ctx:
            pools = make_pools(tc, ctx)
            if n_iters is None:
                build_body(nc, tc, pools, io, phases)
            else:
                with tc.For_i(0, n_iters, 1):
                    build_body(nc, tc, pools, io, phases)
    nc.compile()
    return nc, io


def host_inputs(x, W_qkv, b_qkv, W_out, b_out):
    """Per-core in_maps + the host-side unshard constant."""
    x = np.asarray(x, dtype=np.float32)
    W_qkv = np.asarray(W_qkv, dtype=np.float32)
    b_qkv = np.asarray(b_qkv, dtype=np.float32)
    W_out = np.asarray(W_out, dtype=np.float32)
    b_out = np.asarray(b_out, dtype=np.float32)

    Wq, Wk, Wv = W_qkv[:, 0:D], W_qkv[:, D:2 * D], W_qkv[:, 2 * D:3 * D]
    bq, bk, bv = b_qkv[0:D], b_qkv[D:2 * D], b_qkv[2 * D:3 * D]
    scale = 1.0 / np.sqrt(DH)

    # shared constants
    masks = np.zeros((2, KC, QT), np.float32)
    for r in range(2):
        kk = np.arange(KC)[:, None]
        qq = np.arange(QT)[None, :]
        masks[r] = (qq >= kk + KC * r).astype(np.float32)
    ident = np.eye(128, dtype=np.float32)
    sel = np.zeros((128, 128), np.float32)
    sel[64, 0:64] = 1.0
    ones2 = np.ones((128, HPG), np.float32)

    per_group = []
    for g in range(G):
        cols = []
        bcols = []
        for p in range(NPAIR):
            h0 = g * HPG + 2 * p
            h1 = h0 + 1
            cols.append(Wq[:, h0 * DH:(h0 + 2) * DH] * scale)   # q-pair
            cols.append(Wk[:, h0 * DH:(h0 + 2) * DH])           # k-pair
            bcols.append(bq[h0 * DH:(h0 + 2) * DH] * scale)
            bcols.append(bk[h0 * DH:(h0 + 2) * DH])
        wqk_g = np.concatenate(cols, axis=1)                    # [768, 768]
        bqk_g = np.stack(bcols, axis=1)                         # [128, 6]
        wv_g = Wv[:, g * HPG * DH:(g + 1) * HPG * DH]           # [768, 384]
        wo_g = W_out[g * HPG * DH:(g + 1) * HPG * DH, :]        # [384, 768]
        per_group.append((wqk_g, bqk_g, wv_g, wo_g))

    in_maps = []
    for core in range(N_CORES):
        b, g = core // G, core % G
        wqk_g, bqk_g, wv_g, wo_g = per_group[g]
        in_maps.append(dict(
            x=np.ascontiguousarray(x[b]),
            wqk=np.ascontiguousarray(wqk_g),
            bqk2=np.ascontiguousarray(bqk_g),
            wv=np.ascontiguousarray(wv_g),
            wo=np.ascontiguousarray(wo_g),
            masks=masks, ident=ident, sel=sel,
            ones2=ones2,
        ))
    cvec = (bv @ W_out + b_out).astype(np.float32)              # [768]
    return in_maps, cvec


_CACHE = {}


def kernel(x, W_qkv, b_qkv, W_out, b_out):
    from concourse.bass_utils import run_bass_kernel_spmd

    if "nc" not in _CACHE:
        _CACHE["nc"], _ = build_nc()
    nc = _CACHE["nc"]
    in_maps, cvec = host_inputs(x, W_qkv, b_qkv, W_out, b_out)
    res = run_bass_kernel_spmd(nc, in_maps, list(range(N_CORES)))
    out = np.empty((B, S, D), np.float32)
    for b in range(B):
        out[b] = res.results[2 * b]["out"] + res.results[2 * b + 1]["out"] + cvec
    return out



# revision 23
# speedup vs baseline: 1.2225x; 1.2225x over previous
"""Causal self-attention (B=4, S=2048, D=768, H=12) on 8 TRN2 NeuronCores.

Sharding: batch (4) x head-group (2) = 8 cores.  Each core computes, for its
batch b and 6 heads:
  - x^T supplied pre-transposed by the host (one [768, S] DMA slab per core,
    sliced per 512-column q-tile so compute starts after ~1.5MB)
  - QK^T projection directly in transposed layout (head-dim on partitions),
    Q pre-scaled by 1/sqrt(dh), output cast to bf16
  - V projection in natural layout (bf16), with a ones column appended per
    head (so the AV matmul also produces softmax denominators for free)
  - flash-style causal attention with scores kept transposed (S^T = K Q^T),
    bf16 operands (exact causal chunk widths; no fp32r >=256 free-dim
    widening needed), exp on ACT -> bf16 probs, causal triangle mask as a
    0/1 bf16 multiply on the leading 128 columns of diagonal chunks only
  - AV^T accumulated in PSUM over key chunks -> O^T [dh, q] per head,
    normalized by PE-broadcast reciprocal of the fused sums row
  - partial output projection (its 384 rows of W_out, bf16)
Everything is emitted t-major (projection -> attention -> output projection
per 512-query tile) so the PE-heavy projections of tile t+1 overlap the
ACT-heavy softmax of tile t.
Host: sums the two partial outputs per batch and adds the constant
b_v @ W_out + b_out (V-bias commutes through softmax-normalized attention).
"""

import numpy as np

import concourse.bass as bass
import concourse.tile as tile
import concourse.mybir as mybir
from concourse import bacc
from concourse._compat import with_exitstack  # noqa: F401  (parity with repo kernels)

F32 = mybir.dt.float32
F32R = mybir.dt.float32r
BF16 = mybir.dt.bfloat16

B, S, D = 4, 2048, 768
H, DH = 12, 64
G = 2                 # head groups (tensor-parallel dimension)
HPG = H // G          # heads per group = 6
NPAIR = HPG // 2      # head pairs per group = 3
N_CORES = 8
ST = 128              # S-tile for projections / output rows
QT = 512              # q-tile for attention
KC = 128              # key chunk
N_ST = S // ST        # 16
N_QT = S // QT        # 4
DC = D // 128         # 6 contraction chunks over D


def declare_io(nc):
    """DRAM tensors; names must match in_maps keys."""
    io = {}
    io["xt"] = nc.dram_tensor("xt", [DC, 128, S], F32R, kind="ExternalInput")
    io["wqk"] = nc.dram_tensor("wqk", [DC, 128, 768], F32R, kind="ExternalInput")
    io["bqk2"] = nc.dram_tensor("bqk2", [128, 6], F32, kind="ExternalInput")
    io["wv"] = nc.dram_tensor("wv", [DC, 128, 384], F32R, kind="ExternalInput")
    io["wo"] = nc.dram_tensor("wo", [NPAIR, 128, 768], BF16, kind="ExternalInput")
    io["mask"] = nc.dram_tensor("mask", [KC, KC], BF16, kind="ExternalInput")
    io["sel"] = nc.dram_tensor("sel", [128, 128], F32R, kind="ExternalInput")
    io["out"] = nc.dram_tensor("out", [S, D], F32, kind="ExternalOutput")
    return io


def build_body(nc, tc, pools, io, phases=(1, 2, 3, 4, 5)):
    """Emit one full forward pass (per-core program), t-major."""
    (consts, wqk_p, wv_p, wo_p, xt_p, qkT_p, vsb_p, apr_p, psA, psB, scp,
     pT_p, rcp_p, bc_p, atmp_p, outsb_p) = pools

    Exp = mybir.ActivationFunctionType.Exp
    Copy = mybir.ActivationFunctionType.Copy

    # ---- big tiles ----
    xT = xt_p.tile([128, DC, S], F32R, tag="xt")
    wqk_t = wqk_p.tile([128, DC, 768], F32R, tag="wqk")
    wv_t = wv_p.tile([128, DC, 384], F32R, tag="wv")
    wo_t = wo_p.tile([128, NPAIR, 768], BF16, tag="wo")
    qkT = [qkT_p.tile([128, S], BF16, tag="qkT", name=f"qkT{j}") for j in range(6)]
    apair = [apr_p.tile([128, S], BF16, tag="apr", name=f"apair{p}")
             for p in range(NPAIR)]
    vsb = [vsb_p.tile([128, HPG, 65], BF16, tag="vsb", name=f"vsb{s}")
           for s in range(N_ST)]

    # ---- DMA issue order: first q-tile's x columns + wqk first ----
    def dma_xt(t):
        nc.sync.dma_start(out=xT[:, :, t * QT:(t + 1) * QT],
                          in_=io["xt"][:, :, t * QT:(t + 1) * QT]
                          .rearrange("c p n -> p c n"))

    dma_xt(0)
    nc.sync.dma_start(out=wqk_t, in_=io["wqk"][:].rearrange("c p n -> p c n"))
    bqk2_t = consts.tile([128, 6], F32, tag="bqk2")
    nc.sync.dma_start(out=bqk2_t, in_=io["bqk2"][:])
    mask_t = consts.tile([KC, KC], BF16, tag="mask")
    nc.sync.dma_start(out=mask_t, in_=io["mask"][:])
    sel_t = consts.tile([128, 128], F32R, tag="sel")
    nc.sync.dma_start(out=sel_t, in_=io["sel"][:])
    nc.sync.dma_start(out=wv_t, in_=io["wv"][:].rearrange("c p n -> p c n"))
    dma_xt(1)
    nc.sync.dma_start(out=wo_t, in_=io["wo"][:].rearrange("c p n -> p c n"))
    dma_xt(2)
    dma_xt(3)

    # warm the exp table while DMAs stream
    warm = consts.tile([1, 8], F32, tag="warm")
    nc.vector.memset(warm[0:1, 0:4], 0.0)
    nc.scalar.activation(warm[0:1, 4:8], warm[0:1, 0:4], Exp)

    # ones columns for the fused denominators
    for s in range(N_ST):
        nc.gpsimd.memset(vsb[s][:, :, 64:65], 1.0)

    bqk2_bc = [bqk2_t[:, j:j + 1] for j in range(6)]
    mask_bc = bass.AP(tensor=mask_t.tensor, offset=mask_t.offset,
                      ap=[list(mask_t.ap[0]), [0, 2], list(mask_t.ap[1])])

    for t in range(N_QT):
        # ---- QK^T projection for this q-tile ----
        if 2 in phases:
            for j in range(6):
                pp = psA.tile([128, QT], F32, tag="ps1")
                for c in range(DC):
                    nc.tensor.matmul(pp, wqk_t[:, c, j * 128:(j + 1) * 128],
                                     xT[:, c, t * QT:(t + 1) * QT],
                                     start=(c == 0), stop=(c == DC - 1))
                nc.vector.tensor_scalar_add(qkT[j][:, t * QT:(t + 1) * QT], pp,
                                            bqk2_bc[j])
        # ---- V projection for this q-tile's 4 key chunks ----
        if 3 in phases:
            for s in range(4 * t, 4 * t + 4):
                vp = psA.tile([128, 384], F32, tag="ps1")
                for c in range(DC):
                    nc.tensor.matmul(vp, xT[:, c, s * ST:(s + 1) * ST],
                                     wv_t[:, c, :],
                                     start=(c == 0), stop=(c == DC - 1))
                nc.vector.tensor_copy(vsb[s][:, :, 0:64],
                                      vp[:].rearrange("p (h d) -> p h d", h=HPG))
        # ---- attention for each head pair ----
        if 4 in phases:
            n_kc = 4 * t + 4
            for p in range(NPAIR):
                qp = qkT[2 * p]
                kp = qkT[2 * p + 1]
                av2 = psB.tile([65, 2, QT], F32, tag="ps2")
                pending = None  # (kc, off, w, pt2) awaiting AV emission

                def emit_av(kc, off, w, pt2):
                    for j in (0, 1):
                        nc.tensor.matmul(
                            av2[:, j, off:off + w], vsb[kc][:, 2 * p + j, :],
                            pt2[:, j, 0:w],
                            start=(kc == 0), stop=(kc == n_kc - 1))

                for kc in range(n_kc):
                    r = kc - 4 * t
                    off = 0 if r < 0 else 128 * r
                    w = QT - off
                    sc2 = scp.tile([KC, 2, QT], F32, tag="sc2")
                    pt2 = pT_p.tile([KC, 2, QT], BF16, tag="pT")
                    for j in (0, 1):
                        nc.tensor.matmul(
                            sc2[:, j, 0:w],
                            kp[j * 64:(j + 1) * 64, kc * KC:(kc + 1) * KC],
                            qp[j * 64:(j + 1) * 64, t * QT + off:(t + 1) * QT],
                            start=True, stop=True, tile_position=(j * 64, 0))
                    nc.scalar.activation(pt2[:, :, 0:w], sc2[:, :, 0:w], Exp)
                    if r >= 0:
                        nc.vector.tensor_mul(pt2[:, :, 0:KC], pt2[:, :, 0:KC],
                                             mask_bc)
                    if pending is not None:
                        emit_av(*pending)
                    pending = (kc, off, w, pt2)
                emit_av(*pending)

                # normalize: reciprocal of the fused sums rows (one op for
                # both heads), PE sel-broadcast down the partitions, ACT
                # copies to SBUF, DVE multiplies
                rc = rcp_p.tile([65, 2, QT], F32R, tag="rcp")
                nc.vector.reciprocal(rc[64:65, :, :], av2[64:65, :, :])
                bc_sb = bc_p.tile([64, 2, QT], F32, tag="bcsb")
                for j in (0, 1):
                    bcp = psA.tile([64, QT], F32, tag="ps1")
                    nc.tensor.matmul(bcp, sel_t[64:65, 0:64], rc[64:65, j, :],
                                     start=True, stop=True)
                    nc.scalar.activation(bc_sb[:, j, :], bcp, Copy)
                nc.vector.tensor_mul(apair[p][0:64, t * QT:(t + 1) * QT],
                                     av2[0:64, 0, :], bc_sb[:, 0, :])
                at = atmp_p.tile([64, QT], BF16, tag="atmp")
                nc.vector.tensor_mul(at, av2[0:64, 1, :], bc_sb[:, 1, :])
                nc.sync.dma_start(out=apair[p][64:128, t * QT:(t + 1) * QT],
                                  in_=at)
        # ---- output projection for this q-tile's 4 row tiles ----
        if 5 in phases:
            for s in range(4 * t, 4 * t + 4):
                o1 = psA.tile([128, 512], F32, tag="ps1")
                o2 = psA.tile([128, 256], F32, tag="ps1")
                for p in range(NPAIR):
                    nc.tensor.matmul(o1, apair[p][:, s * ST:(s + 1) * ST],
                                     wo_t[:, p, 0:512],
                                     start=(p == 0), stop=(p == NPAIR - 1))
                for p in range(NPAIR):
                    nc.tensor.matmul(o2, apair[p][:, s * ST:(s + 1) * ST],
                                     wo_t[:, p, 512:768],
                                     start=(p == 0), stop=(p == NPAIR - 1))
                osb = outsb_p.tile([128, D], F32, tag="outsb")
                nc.vector.tensor_copy(osb[:, 0:512], o1)
                nc.vector.tensor_copy(osb[:, 512:768], o2)
                nc.sync.dma_start(out=io["out"][s * ST:(s + 1) * ST, :], in_=osb)


def make_pools(tc, ctx):
    consts = ctx.enter_context(tc.tile_pool(name="consts", bufs=1))
    wqk_p = ctx.enter_context(tc.tile_pool(name="wqk", bufs=1))
    wv_p = ctx.enter_context(tc.tile_pool(name="wv", bufs=1))
    wo_p = ctx.enter_context(tc.tile_pool(name="wo", bufs=1))
    xt_p = ctx.enter_context(tc.tile_pool(name="xt", bufs=1))
    qkT_p = ctx.enter_context(tc.tile_pool(name="qkT", bufs=6))
    vsb_p = ctx.enter_context(tc.tile_pool(name="vsb", bufs=16))
    apr_p = ctx.enter_context(tc.tile_pool(name="apr", bufs=3))
    psA = ctx.enter_context(tc.tile_pool(name="psA", bufs=2, space="PSUM"))
    psB = ctx.enter_context(tc.tile_pool(name="psB", bufs=1, space="PSUM"))
    scp = ctx.enter_context(tc.tile_pool(name="scp", bufs=2, space="PSUM"))
    pT_p = ctx.enter_context(tc.tile_pool(name="pT", bufs=3))
    rcp_p = ctx.enter_context(tc.tile_pool(name="rcp", bufs=2))
    bc_p = ctx.enter_context(tc.tile_pool(name="bc", bufs=2))
    atmp_p = ctx.enter_context(tc.tile_pool(name="atmp", bufs=2))
    outsb_p = ctx.enter_context(tc.tile_pool(name="outsb", bufs=2))
    return (consts, wqk_p, wv_p, wo_p, xt_p, qkT_p, vsb_p, apr_p, psA, psB,
            scp, pT_p, rcp_p, bc_p, atmp_p, outsb_p)


def build_nc(n_iters=None, phases=(1, 2, 3, 4, 5)):
    """Build the per-core program. n_iters wraps the body in a HW loop
    (timing harness only; the graded path uses n_iters=None)."""
    from contextlib import ExitStack

    nc = bacc.Bacc(trn_type="TRN2", debug=False)
    nc._allow_low_precision_reason = "bf16 attention path stays within rel tol"
    io = declare_io(nc)
    with tile.TileContext(nc) as tc:
        with ExitStack() as ctx:
            pools = make_pools(tc, ctx)
            if n_iters is None:
                build_body(nc, tc, pools, io, phases)
            else:
                with tc.For_i(0, n_iters, 1):
                    build_body(nc, tc, pools, io, phases)
    nc.compile()
    return nc, io


def host_inputs(x, W_qkv, b_qkv, W_out, b_out):
    """Per-core in_maps + the host-side unshard constant."""
    x = np.asarray(x, dtype=np.float32)
    W_qkv = np.asarray(W_qkv, dtype=np.float32)
    b_qkv = np.asarray(b_qkv, dtype=np.float32)
    W_out = np.asarray(W_out, dtype=np.float32)
    b_out = np.asarray(b_out, dtype=np.float32)

    Wq, Wk, Wv = W_qkv[:, 0:D], W_qkv[:, D:2 * D], W_qkv[:, 2 * D:3 * D]
    bq, bk, bv = b_qkv[0:D], b_qkv[D:2 * D], b_qkv[2 * D:3 * D]
    scale = 1.0 / np.sqrt(DH)

    # shared constants
    import jax.numpy as jnp
    kk = np.arange(KC)[:, None]
    qq = np.arange(KC)[None, :]
    mask = (qq >= kk).astype(np.float32)
    mask_bf = np.asarray(jnp.asarray(mask, dtype=jnp.bfloat16))
    sel = np.zeros((128, 128), np.float32)
    sel[64, 0:64] = 1.0
    xtb = [np.ascontiguousarray(x[b].T).reshape(DC, 128, S) for b in range(B)]

    per_group = []
    for g in range(G):
        cols = []
        bcols = []
        for p in range(NPAIR):
            h0 = g * HPG + 2 * p
            cols.append(Wq[:, h0 * DH:(h0 + 2) * DH] * scale)   # q-pair
            cols.append(Wk[:, h0 * DH:(h0 + 2) * DH])           # k-pair
            bcols.append(bq[h0 * DH:(h0 + 2) * DH] * scale)
            bcols.append(bk[h0 * DH:(h0 + 2) * DH])
        wqk_g = np.concatenate(cols, axis=1).reshape(DC, 128, 768)
        bqk_g = np.stack(bcols, axis=1)                         # [128, 6]
        wv_g = Wv[:, g * HPG * DH:(g + 1) * HPG * DH].reshape(DC, 128, 384)
        wo_g = W_out[g * HPG * DH:(g + 1) * HPG * DH, :].reshape(NPAIR, 128, 768)
        wo_bf = np.asarray(jnp.asarray(wo_g, dtype=jnp.bfloat16))
        per_group.append((wqk_g, bqk_g, wv_g, wo_bf))

    in_maps = []
    for core in range(N_CORES):
        b, g = core // G, core % G
        wqk_g, bqk_g, wv_g, wo_bf = per_group[g]
        in_maps.append(dict(
            xt=xtb[b],
            wqk=np.ascontiguousarray(wqk_g),
            bqk2=np.ascontiguousarray(bqk_g),
            wv=np.ascontiguousarray(wv_g),
            wo=np.ascontiguousarray(wo_bf),
            mask=mask_bf, sel=sel,
        ))
    cvec = (bv @ W_out + b_out).astype(np.float32)              # [768]
    return in_maps, cvec


_CACHE = {}


def kernel(x, W_qkv, b_qkv, W_out, b_out):
    from concourse.bass_utils import run_bass_kernel_spmd

    if "nc" not in _CACHE:
        _CACHE["nc"], _ = build_nc()
    nc = _CACHE["nc"]
    in_maps, cvec = host_inputs(x, W_qkv, b_qkv, W_out, b_out)
    res = run_bass_kernel_spmd(nc, in_maps, list(range(N_CORES)))
    out = np.empty((B, S, D), np.float32)
    for b in range(B):
        out[b] = res.results[2 * b]["out"] + res.results[2 * b + 1]["out"] + cvec
    return out
